# revision 3
# baseline (speedup 1.0000x reference)
"""3-layer GCN (PyG GCNConv semantics) on 8 Trainium2 NeuronCores.

Strategy (graph/data parallel, per the node-range sharding hint):
  - Nodes sharded by contiguous range across 8 cores (targets + their rows).
  - Symmetric norm factored: dinv[col]*ew*dinv[row]; dinv folded into the
    gather tables (rows pre-scaled by dinv[row]) and the post-accumulation
    scale (dinv[col]); self-loops appended as ordinary edges with ew=1.
  - Degree via fixed-slot padded DVE reduction; dinv = rsqrt (ACT).
  - Per layer: gather source rows from an HBM table (dma_gather, f16 256B
    rows, 4 source-quadrant passes to satisfy int16 indices), build a
    per-chunk selection matrix S[e, ord] = ew (one fused DVE op against a
    static iota), reduce with PE matmuls into per-128-target PSUM pairs,
    accumulate quadrant passes in SBUF, then fused post (x dinv, +b, tanh).
  - Dense W matmuls run per-shard between layers using an f16 DMA-transpose
    round trip through HBM; tables are AllGathered across cores.
"""

import numpy as np

import concourse.bacc as bacc
import concourse.bass as bass
import concourse.mybir as mybir
import concourse.tile as tile
from concourse.bass_utils import run_bass_kernel_spmd
from concourse.vector_clock import ScopedClock, VectorClock

P = 8          # cores
WIN = 64       # targets per window (selection-matrix ordinal space)
GCALL_MAX = 8192

F16 = mybir.dt.float16
F32 = mybir.dt.float32
I16 = mybir.dt.int16


# --- walrus in this container rejects >1 sem wait on the tail drain; split it ---
_MAXW = 1


def _drain_and_barrier_split(self, tick_clock, wait_clock):
    gc = tick_clock.global_clock
    n = len(gc)
    live = [i for i in range(n) if gc[i] > 0]
    for s0 in range(0, max(len(live), 1), _MAXW):
        keep = set(live[s0:s0 + _MAXW])
        sub = VectorClock([gc[i] if i in keep else 0 for i in range(n)])
        d = self.nc.sync.drain()
        wait_clock.add_sem_waits(d.ins, ScopedClock({None: sub}))
    self.nc.all_engine_barrier()
    popped = self.nc._tile_sem_poison_stack.pop()
    assert popped is self._sem_poison
    self.nc.clear_and_free_semaphores(list(self.sems.allocated().values()))
    self.nc.all_engine_barrier()


tile.TileContext._drain_and_barrier = _drain_and_barrier_split


def _host_prep(x, edge_index, edge_weight):
    """Integer-only sharding/layout prep. Returns per-core input maps' arrays."""
    n, nf = x.shape
    assert n % P == 0
    shard = n // P
    shard_pad = -(-shard // 128) * 128
    npairs = shard_pad // 128
    nwin = shard_pad // WIN
    qrows = 2 * shard_pad            # rows per quadrant table (2 cores)
    assert qrows <= 32767

    row = edge_index[0].astype(np.int64)
    col = edge_index[1].astype(np.int64)
    ew = edge_weight.astype(np.float32)
    # self loops
    loops = np.arange(n, dtype=np.int64)
    row = np.concatenate([row, loops])
    col = np.concatenate([col, loops])
    ew = np.concatenate([ew, np.ones(n, np.float32)])

    core = col // shard
    lcol = col - core * shard
    win = lcol // WIN
    ordi = lcol % WIN
    row_pad = (row // shard) * shard_pad + (row % shard)
    quad = row_pad // qrows
    lrow = (row_pad - quad * qrows).astype(np.int16)

    # group edges by (core, quad, window)
    key = (core * 4 + quad) * nwin + win
    order = np.argsort(key, kind="stable")
    key_s = key[order]
    counts = np.bincount(key_s, minlength=P * 4 * nwin)
    starts = np.zeros(P * 4 * nwin + 1, np.int64)
    np.cumsum(counts, out=starts[1:])
    cmax = max(1, int(-(-counts.max() // 128)))
    S = cmax * 128                          # slots per (core, quad, window)
    pos = np.arange(len(key_s)) - starts[key_s]
    slot = key_s * S + pos                  # global slot id

    tot = P * 4 * nwin * S
    lrow_sl = np.zeros(tot, np.int16)
    ew_sl = np.zeros(tot, np.float32)
    ord_sl = np.zeros(tot, np.float32)
    lrow_sl[slot] = lrow[order]
    ew_sl[slot] = ew[order]
    ord_sl[slot] = ordi[order].astype(np.float32)

    lrow_sl = lrow_sl.reshape(P, 4, nwin, S)
    ew_sl = ew_sl.reshape(P, 4, nwin, S)
    ord_sl = ord_sl.reshape(P, 4, nwin, S)

    # windows per gather call: even, divides nwin, G <= GCALL_MAX
    wpc = None
    for cand in (14, 28, 4, 2):
        if nwin % cand == 0 and cand * S <= GCALL_MAX:
            wpc = cand
            break
    assert wpc is not None, (nwin, S)
    G = wpc * S
    ncalls = nwin // wpc

    # gather index stream, wrapped [16, G//16] per call, pre-replicated to 128
    idx4 = lrow_sl.reshape(P, 4, ncalls, G)
    i = np.arange(G)
    idx_w = np.zeros((P, 4, ncalls, 16, G // 16), np.int16)
    idx_w[:, :, :, i % 16, i // 16] = idx4
    idx_strm = np.broadcast_to(
        idx_w[:, :, :, None, :, :], (P, 4, ncalls, 8, 16, G // 16)
    ).reshape(P, 4 * ncalls * 128, G // 16).copy()

    # ew / ord streams in chunk-lane layout: [core, 128, nchunks_total]
    nch = 4 * nwin * cmax
    ew_cl = np.ascontiguousarray(
        ew_sl.reshape(P, 4 * nwin * cmax, 128).transpose(0, 2, 1)
    ).astype(np.float32)
    ord_cl = np.ascontiguousarray(
        ord_sl.reshape(P, 4 * nwin * cmax, 128).transpose(0, 2, 1)
    ).astype(np.float32)

    # degree slots: target t = g*128 + p  (g in [0, npairs), p in [0,128))
    tkey = col  # global target
    tcore = core
    tloc = lcol
    dcounts = np.bincount(tcore * shard + tloc, minlength=P * shard)
    degpad = max(8, int(-(-dcounts.max() // 4) * 4))
    torder = np.argsort(tcore * shard + tloc, kind="stable")
    tks = (tcore * shard + tloc)[torder]
    tstarts = np.zeros(P * shard + 1, np.int64)
    np.cumsum(np.bincount(tks, minlength=P * shard), out=tstarts[1:])
    tpos = np.arange(len(tks)) - tstarts[tks]
    ew_deg = np.zeros((P, shard_pad, degpad), np.float32)
    c_t, l_t = tks // shard, tks % shard
    ew_deg[c_t, l_t, tpos] = ew[torder]
    # fake targets (shard..shard_pad): one dummy slot ew=1 -> deg=1, dinv=1
    if shard_pad > shard:
        ew_deg[:, shard:, 0] = 1.0
    # reshape to [core, 128, npairs*degpad] with t = g*128+p
    ew_deg = np.ascontiguousarray(
        ew_deg.reshape(P, npairs, 128, degpad).transpose(0, 2, 1, 3)
    ).reshape(P, 128, npairs * degpad)

    # x transposed, padded, f16
    xp = np.zeros((P, shard_pad, nf), np.float32)
    xp[:, :shard, :] = x.reshape(P, shard, nf)
    xT = np.ascontiguousarray(xp.transpose(0, 2, 1)).astype(np.float16)

    meta = dict(shard=shard, shard_pad=shard_pad, npairs=npairs, nwin=nwin,
                qrows=qrows, cmax=cmax, S=S, wpc=wpc, G=G, ncalls=ncalls,
                nch=nch, degpad=degpad)
    return meta, idx_strm, ew_cl, ord_cl, ew_deg, xT


def _build(meta, nf, h1, h2, ncls):
    m = meta
    shard_pad, npairs, nwin = m["shard_pad"], m["npairs"], m["nwin"]
    qrows, cmax, G, wpc, ncalls = m["qrows"], m["cmax"], m["G"], m["wpc"], m["ncalls"]
    nch, degpad = m["nch"], m["degpad"]
    cpc = wpc * cmax                       # chunks per call
    ppc = wpc // 2                         # pairs per call
    ntab = 4 * qrows                       # padded total rows

    nc = bacc.Bacc("TRN2", target_bir_lowering=False, num_devices=P,
                   dynamic_dma_scratch_size=32768)

    # inputs
    xT_d = nc.dram_tensor("xT", [nf, shard_pad], F16, kind="ExternalInput")
    idx_d = nc.dram_tensor("idx_strm", [4 * ncalls * 128, G // 16], I16, kind="ExternalInput")
    ew_d = nc.dram_tensor("ew_cl", [128, nch], F32, kind="ExternalInput")
    ord_d = nc.dram_tensor("ord_cl", [128, nch], F32, kind="ExternalInput")
    ewdeg_d = nc.dram_tensor("ew_deg", [128, npairs * degpad], F32, kind="ExternalInput")
    w1_d = nc.dram_tensor("W1", [nf, h1], F16, kind="ExternalInput")
    w2_d = nc.dram_tensor("W2", [h1, h2], F16, kind="ExternalInput")
    w3_d = nc.dram_tensor("W3", [h2, ncls], F16, kind="ExternalInput")
    b1_d = nc.dram_tensor("b1", [128, h1], F32, kind="ExternalInput")
    b2_d = nc.dram_tensor("b2", [128, h2], F32, kind="ExternalInput")
    b3_d = nc.dram_tensor("b3", [128, ncls], F32, kind="ExternalInput")
    iota_d = nc.dram_tensor("iota64", [128, WIN], F16, kind="ExternalInput")

    emb_d = nc.dram_tensor("emb", [shard_pad, h2], F32, kind="ExternalOutput")
    logit_d = nc.dram_tensor("logits", [shard_pad, ncls], F32, kind="ExternalOutput")

    # internal DRAM
    shard_t = [nc.dram_tensor(f"tshard{i}", [shard_pad, 128], F16) for i in range(3)]
    full_t = [nc.dram_tensor(f"tfull{i}", [ntab, 128], F16, addr_space="Shared")
              for i in range(3)]
    h_d = [nc.dram_tensor(f"hbuf{i}", [shard_pad, 128], F16) for i in range(2)]

    groups = [list(range(P))]

    with tile.TileContext(nc) as tc:
        with (
            tc.tile_pool(name="persist", bufs=1) as pp,
            tc.tile_pool(name="sb", bufs=3) as sb,
            tc.tile_pool(name="msgp", bufs=2) as msgp,
            tc.tile_pool(name="sp", bufs=4) as spool,
            tc.tile_pool(name="psum", bufs=4, space="PSUM") as ps,
            tc.tile_pool(name="psd", bufs=2, space="PSUM") as psd,
        ):
            # ---- persistent loads ----
            xT_sb = pp.tile([nf, shard_pad], F16)
            nc.sync.dma_start(out=xT_sb[:], in_=xT_d[:, :])
            ew_sb = pp.tile([128, nch], F32)
            nc.sync.dma_start(out=ew_sb[:], in_=ew_d[:, :])
            ord_sb = pp.tile([128, nch], F32)
            nc.sync.dma_start(out=ord_sb[:], in_=ord_d[:, :])
            w1_sb = pp.tile([nf, h1], F16)
            nc.sync.dma_start(out=w1_sb[:], in_=w1_d[:, :])
            w2_sb = pp.tile([h1, h2], F16)
            nc.sync.dma_start(out=w2_sb[:], in_=w2_d[:, :])
            w3_sb = pp.tile([h2, ncls], F16)
            nc.sync.dma_start(out=w3_sb[:], in_=w3_d[:, :])
            b1_sb = pp.tile([128, h1], F32)
            nc.sync.dma_start(out=b1_sb[:], in_=b1_d[:, :])
            b2_sb = pp.tile([128, h2], F32)
            nc.sync.dma_start(out=b2_sb[:], in_=b2_d[:, :])
            b3_sb = pp.tile([128, ncls], F32)
            nc.sync.dma_start(out=b3_sb[:], in_=b3_d[:, :])
            iota_sb = pp.tile([128, WIN], F16)
            nc.sync.dma_start(out=iota_sb[:], in_=iota_d[:, :])
            dinv_sb = pp.tile([128, npairs], F32)
            acc_sb = pp.tile([128, npairs * h1], F32)

            # ---- degree -> dinv ----
            DB = 14 if npairs % 14 == 0 else (7 if npairs % 7 == 0 else 1)
            deg_sb = pp.tile([128, npairs], F32)
            for b0 in range(0, npairs, DB):
                nb = min(DB, npairs - b0)
                dt_ = sb.tile([128, nb * degpad], F32, tag="degload")
                nc.sync.dma_start(out=dt_[:], in_=ewdeg_d[:, b0 * degpad:(b0 + nb) * degpad])
                nc.vector.tensor_reduce(
                    out=deg_sb[:, b0:b0 + nb],
                    in_=dt_[:].rearrange("p (g s) -> p g s", s=degpad),
                    axis=mybir.AxisListType.X, op=mybir.AluOpType.add)
            rec_sb = pp.tile([128, npairs], F32)
            nc.vector.reciprocal(out=rec_sb[:], in_=deg_sb[:])
            nc.scalar.activation(out=dinv_sb[:], in_=rec_sb[:],
                                 func=mybir.ActivationFunctionType.Sqrt)

            def dense_to_table(lhsT_sb, w_sb, k, d_out, tab, scale_by_dinv=True):
                # table rows [128, 128] f16: cols 0:d_out = dinv * (h @ W)
                for s in range(npairs):
                    pt = psd.tile([128, d_out], F32, tag="dpsum")
                    nc.tensor.matmul(out=pt[:], lhsT=lhsT_sb[:k, s * 128:(s + 1) * 128],
                                     rhs=w_sb[:k, :d_out], start=True, stop=True)
                    tt = sb.tile([128, 128], F16, tag="trow")
                    nc.vector.memset(tt[:, d_out:], 0.0)
                    nc.vector.tensor_scalar_mul(out=tt[:, :d_out], in0=pt[:],
                                                scalar1=dinv_sb[:, s:s + 1])
                    nc.sync.dma_start(out=tab[s * 128:(s + 1) * 128, :], in_=tt[:])

            # ---- dense-1: table1 = dinv * (x @ W1) ----
            dense_to_table(xT_sb, w1_sb, nf, h1, shard_t[0])
            nc.gpsimd.collective_compute(
                "AllGather", mybir.AluOpType.bypass, replica_groups=groups,
                ins=[shard_t[0].ap().opt()], outs=[full_t[0].ap().opt()])

            def edge_layer(li, d_io, post):
                """Gather+reduce layer: acc = A_ew @ table_li ; then post(pair, acc_ap)."""
                tab = full_t[li]
                for q in range(4):
                    base = q * qrows
                    for cidx in range(ncalls):
                        it = spool.tile([128, G // 16], I16, tag="idx")
                        r0 = (q * ncalls + cidx) * 128
                        nc.sync.dma_start(out=it[:], in_=idx_d[r0:r0 + 128, :])
                        mt = msgp.tile([128, G // 128, 128], F16, tag="msg")
                        nc.gpsimd.dma_gather(
                            out_ap=mt[:], in_ap=tab[base:base + qrows, :],
                            idxs_ap=it[:], num_idxs=G, num_idxs_reg=G,
                            elem_size=128, single_packet=False)
                        ch0 = (q * nwin + cidx * wpc) * cmax
                        for pi in range(ppc):
                            pt = ps.tile([128, d_io], F32, tag="epsum")
                            pair = cidx * ppc + pi
                            for h in range(2):
                                for ci in range(cmax):
                                    cc = ch0 + (pi * 2 + h) * cmax + ci
                                    lc = (pi * 2 + h) * cmax + ci
                                    s_t = spool.tile([128, WIN], F16, tag="sdyn")
                                    nc.vector.scalar_tensor_tensor(
                                        out=s_t[:], in0=iota_sb[:],
                                        scalar=ord_sb[:, cc:cc + 1],
                                        in1=ew_sb[:, cc:cc + 1].to_broadcast([128, WIN]),
                                        op0=mybir.AluOpType.is_equal,
                                        op1=mybir.AluOpType.mult)
                                    nc.tensor.matmul(
                                        out=pt[WIN * h:WIN * (h + 1), :],
                                        lhsT=s_t[:], rhs=mt[:, lc, :d_io],
                                        start=(ci == 0), stop=(ci == cmax - 1))
                            a_ap = acc_sb[:, pair * h1:pair * h1 + d_io]
                            if q == 0:
                                nc.vector.tensor_copy(out=a_ap, in_=pt[:])
                            else:
                                nc.vector.tensor_add(out=a_ap, in0=a_ap, in1=pt[:])
                for pair in range(npairs):
                    post(pair, acc_sb[:, pair * h1:pair * h1 + d_io])

            # ---- layer 1 ----
            def post1(pair, a_ap):
                t = sb.tile([128, h1], F32, tag="post_f32")
                nc.vector.scalar_tensor_tensor(
                    out=t[:], in0=a_ap, scalar=dinv_sb[:, pair:pair + 1],
                    in1=b1_sb[:, :], op0=mybir.AluOpType.mult,
                    op1=mybir.AluOpType.add)
                ht = sb.tile([128, h1], F16, tag="post_f16")
                nc.scalar.activation(out=ht[:], in_=t[:],
                                     func=mybir.ActivationFunctionType.Tanh)
                nc.sync.dma_start(out=h_d[0][pair * 128:(pair + 1) * 128, :h1], in_=ht[:])

            edge_layer(0, h1, post1)

            # ---- dense-2: table2 = dinv * (h1 @ W2) ----
            h1T = pp.tile([h1, shard_pad], F16)
            nc.sync.dma_start_transpose(out=h1T[:], in_=h_d[0][:, :h1])
            dense_to_table(h1T, w2_sb, h1, h2, shard_t[1])
            nc.gpsimd.collective_compute(
                "AllGather", mybir.AluOpType.bypass, replica_groups=groups,
                ins=[shard_t[1].ap().opt()], outs=[full_t[1].ap().opt()])

            # ---- layer 2 ----
            def post2(pair, a_ap):
                t = sb.tile([128, h2], F32, tag="post_f32")
                nc.vector.scalar_tensor_tensor(
                    out=t[:], in0=a_ap, scalar=dinv_sb[:, pair:pair + 1],
                    in1=b2_sb[:, :], op0=mybir.AluOpType.mult,
                    op1=mybir.AluOpType.add)
                nc.sync.dma_start(out=emb_d[pair * 128:(pair + 1) * 128, :], in_=t[:])
                ht = sb.tile([128, h2], F16, tag="post_f16")
                nc.scalar.activation(out=ht[:], in_=t[:],
                                     func=mybir.ActivationFunctionType.Tanh)
                nc.sync.dma_start(out=h_d[1][pair * 128:(pair + 1) * 128, :h2], in_=ht[:])

            edge_layer(1, h2, post2)

            # ---- dense-3: table3 = dinv * (tanh(emb) @ W3) ----
            h2T = pp.tile([h2, shard_pad], F16)
            nc.sync.dma_start_transpose(out=h2T[:], in_=h_d[1][:, :h2])
            dense_to_table(h2T, w3_sb, h2, ncls, shard_t[2])
            nc.gpsimd.collective_compute(
                "AllGather", mybir.AluOpType.bypass, replica_groups=groups,
                ins=[shard_t[2].ap().opt()], outs=[full_t[2].ap().opt()])

            # ---- layer 3 ----
            def post3(pair, a_ap):
                t = sb.tile([128, ncls], F32, tag="post_f32")
                nc.vector.scalar_tensor_tensor(
                    out=t[:], in0=a_ap, scalar=dinv_sb[:, pair:pair + 1],
                    in1=b3_sb[:, :], op0=mybir.AluOpType.mult,
                    op1=mybir.AluOpType.add)
                nc.sync.dma_start(out=logit_d[pair * 128:(pair + 1) * 128, :], in_=t[:])

            edge_layer(2, ncls, post3)

    nc.finalize()
    return nc


_CACHE = {}


def kernel(x, edge_index, edge_weight, W1, b1, W2, b2, W3, b3):
    x = np.asarray(x)
    edge_index = np.asarray(edge_index)
    edge_weight = np.asarray(edge_weight)
    n, nf = x.shape
    h1 = np.asarray(W1).shape[1]
    h2 = np.asarray(W2).shape[1]
    ncls = np.asarray(W3).shape[1]

    meta, idx_strm, ew_cl, ord_cl, ew_deg, xT = _host_prep(x, edge_index, edge_weight)

    ckey = (n, nf, h1, h2, ncls, meta["cmax"], meta["degpad"])
    if ckey not in _CACHE:
        _CACHE[ckey] = _build(meta, nf, h1, h2, ncls)
    nc = _CACHE[ckey]

    iota = np.broadcast_to(np.arange(WIN, dtype=np.float16), (128, WIN)).copy()
    w1f = np.asarray(W1).astype(np.float16)
    w2f = np.asarray(W2).astype(np.float16)
    w3f = np.asarray(W3).astype(np.float16)
    b1f = np.broadcast_to(np.asarray(b1, np.float32), (128, h1)).copy()
    b2f = np.broadcast_to(np.asarray(b2, np.float32), (128, h2)).copy()
    b3f = np.broadcast_to(np.asarray(b3, np.float32), (128, ncls)).copy()

    in_maps = []
    for c in range(P):
        in_maps.append({
            "xT": xT[c], "idx_strm": idx_strm[c], "ew_cl": ew_cl[c],
            "ord_cl": ord_cl[c], "ew_deg": ew_deg[c],
            "W1": w1f, "W2": w2f, "W3": w3f,
            "b1": b1f, "b2": b2f, "b3": b3f, "iota64": iota,
        })
    res = run_bass_kernel_spmd(nc, in_maps, core_ids=list(range(P)))

    shard, shard_pad = meta["shard"], meta["shard_pad"]
    logits = np.concatenate([res.results[c]["logits"][:shard] for c in range(P)], axis=0)
    emb = np.concatenate([res.results[c]["emb"][:shard] for c in range(P)], axis=0)
    return (logits.astype(np.float32), emb.astype(np.float32))
